# revision 5
# baseline (speedup 1.0000x reference)
"""Trainium2 Bass kernel for nn_ComplexMultiheadAttention.

Problem (reference.py): complex multihead attention,
  B=2, N=1024, D=1024, HEADS=16, dim_head=64.
  q/k/v = complex linear projections of x = x_real + i*x_imag,
  4 softmax-attention combos g0..g3 over (q-part, k-part, v-part),
  sign-combined into o_real/o_imag, then a complex output projection.
  Output: [2, B, N, D] fp32 (real, imag).

Sharding (8 NeuronCores): core c = (b = c // 4) x (head group hg = c % 4,
4 heads each). Each core computes projections + attention + sign-combine
for its 4 heads and a partial output projection (its heads' contribution,
full output columns); the host unshards by summing the 4 partials per
batch.

Kernel design (v1, all-bf16):
- Everything in bf16 (fp32 PSUM accumulation): halves DMA + SBUF vs
  fp32r, enables FWL weight loads. Measured numerics: ~9e-3 rel err.
- Host pre-packs every tensor into its exact SBUF layout so all DMAs
  use >=1KB contiguous lines.
- Phase A (projections): q/k computed transposed, head PAIR per 128
  partitions (head A dims on rows 0:64, head B on 64:128) straight out
  of the PSUM chain - no duplication/shift DMAs. x streams on the sync
  DMA queue, weights on the scalar queue so the first LDWEIGHTS never
  queues behind the 4MB x stream. v chains interleaved between q/k
  chains so vhat completes just before the softmax pipeline needs it.
- Phase B (attention): QK^T via K=64 row-packed matmul PAIRS
  (tile_position (0,0)/(64,0) by base partition) - two heads' S^T tiles
  per PE pass, halving QK cost vs duplicated-K128. exp on ScalarE
  (the pacing engine: 155us of ACT) directly out of PSUM; denominator
  via ones-column appended to V; reciprocal+broadcast+mul in bf16
  (2x DVE rate).
- Phase C (output projection): weights preloaded into SBUF at kernel
  start (sync queue, right after x) so the 2MB load fully overlaps
  phases A/B. O kept bf16.
- PSUM budget: phase A chains 2x[128,512] (2 banks) + QK st 2x[128,1024]
  (4 banks) + AV 2x[65,512] (2 banks) = 8 banks, so attention starts
  while projections still run (the old kernel's 8-bank phase-A pool
  serialized the phases).
"""

import numpy as np

import concourse.mybir as mybir
import concourse.tile as tile
from concourse import bacc
from concourse.bass_utils import run_bass_kernel_spmd

P = 128
NTOK = 1024  # tokens per batch
KD = 16  # k-tiles over the stacked 2048 contraction dim
CD = 64  # dim per head
HL = 4  # heads per core
F32 = mybir.dt.float32
BF16 = mybir.dt.bfloat16
BF16NP = mybir.dt.np(mybir.dt.bfloat16)
EXP = mybir.ActivationFunctionType.Exp
SCALE = float(CD) ** -0.5

_nc_cache = None


def _build():
    nc = bacc.Bacc("TRN2", target_bir_lowering=False, debug=False, num_devices=8)

    # all inputs host-packed to exact SBUF layouts
    x = nc.declare_dram_parameter("x", [P, KD, NTOK], BF16, isOutput=False)
    wnames = ["wqr", "wkr", "wqi", "wki"]
    wd = {
        n: nc.declare_dram_parameter(n, [2, P, KD, 128], BF16, isOutput=False)
        for n in wnames
    }  # [pair, p, kt, cols]
    wv = nc.declare_dram_parameter("wv", [P, KD, 512], BF16, isOutput=False)
    wyr = nc.declare_dram_parameter("wyr", [P, HL, NTOK], BF16, isOutput=False)
    wyi = nc.declare_dram_parameter("wyi", [P, HL, NTOK], BF16, isOutput=False)
    yp = nc.declare_dram_parameter("ypart", [2, NTOK, 1024], F32, isOutput=True)

    with tile.TileContext(nc) as tc:
        with (
            tc.tile_pool(name="persist", bufs=1) as pp,
            tc.tile_pool(name="small", bufs=2) as sp,
            tc.tile_pool(name="ps_a", bufs=2, space="PSUM") as psa,
            tc.tile_pool(name="ps_st", bufs=2, space="PSUM") as pst,
            tc.tile_pool(name="ps_av", bufs=2, space="PSUM") as pav,
        ):
            # q^T/k^T: [dims: head A 0:64 | head B 64:128, pair, tok]
            qrT = pp.tile([P, 2, NTOK], BF16, tag="qrT")
            qiT = pp.tile([P, 2, NTOK], BF16, tag="qiT")
            krT = pp.tile([P, 2, NTOK], BF16, tag="krT")
            kiT = pp.tile([P, 2, NTOK], BF16, tag="kiT")
            # V with ones column appended: [ktok-in-tile, jt, head, 65]
            vhat_r = pp.tile([P, 8, HL, CD + 1], BF16, tag="vhr")
            vhat_i = pp.tile([P, 8, HL, CD + 1], BF16, tag="vhi")
            # combined attention output: [o_r dims 0:64 | o_i dims 64:128, head, tok]
            O = pp.tile([P, HL, NTOK], BF16, tag="O")
            wyr_sb = pp.tile([P, HL, NTOK], BF16, tag="wyrs")
            wyi_sb = pp.tile([P, HL, NTOK], BF16, tag="wyis")

            nc.vector.memset(vhat_r[:, :, :, CD : CD + 1], 1.0)
            nc.vector.memset(vhat_i[:, :, :, CD : CD + 1], 1.0)

            with (
                tc.tile_pool(name="pa_x", bufs=1) as pax,
                tc.tile_pool(name="pa_w", bufs=1) as paw,
            ):
                xs = pax.tile([P, KD, NTOK], BF16, tag="xs")
                for kt in range(KD):
                    nc.sync.dma_start(xs[:, kt, :], x[:, kt, :])
                # phase C weights ride the sync queue right after x
                nc.sync.dma_start(wyr_sb[:], wyr[:])
                nc.sync.dma_start(wyi_sb[:], wyi[:])

                def emit_qk(pair, wn, dstT, dmaeng):
                    wt = paw.tile([P, KD, 128], BF16, tag="wt2", name="wt", bufs=2)
                    for k0 in range(0, KD, 4):
                        dmaeng.dma_start(
                            wt[:, k0 : k0 + 4, :], wd[wn][pair, :, k0 : k0 + 4, :]
                        )
                    for tch in range(2):
                        ps = psa.tile([P, 512], F32, tag="pa", name="ps")
                        for kt in range(KD):
                            nc.tensor.matmul(
                                ps[:],
                                wt[:, kt, :],
                                xs[:, kt, tch * 512 : (tch + 1) * 512],
                                start=(kt == 0),
                                stop=(kt == KD - 1),
                            )
                        nc.vector.tensor_copy(
                            dstT[:, pair, tch * 512 : (tch + 1) * 512], ps[:]
                        )

                def emit_v(tts, wt_holder):
                    if wt_holder[0] is None:
                        wt_holder[0] = paw.tile(
                            [P, KD, 512], BF16, tag="wtv", name="wtv", bufs=1
                        )
                        for k0 in range(0, KD, 4):
                            nc.scalar.dma_start(
                                wt_holder[0][:, k0 : k0 + 4, :], wv[:, k0 : k0 + 4, :]
                            )
                    wt = wt_holder[0]
                    for tt in tts:
                        ps = psa.tile([P, 512], F32, tag="pa", name="ps")
                        for kt in range(KD):
                            nc.tensor.matmul(
                                ps[:],
                                xs[:, kt, tt * 128 : (tt + 1) * 128],
                                wt[:, kt, :],
                                start=(kt == 0),
                                stop=(kt == KD - 1),
                            )
                        nc.vector.tensor_copy(
                            vhat_r[:, tt, :, 0:CD],
                            ps[:, 0:256].rearrange("p (h d) -> p h d", d=CD),
                        )
                        nc.vector.tensor_copy(
                            vhat_i[:, tt, :, 0:CD],
                            ps[:, 256:512].rearrange("p (h d) -> p h d", d=CD),
                        )

                # interleave so g0/g2 q,k land early and vhat completes
                # right before the first AV needs it
                vh = [None]
                emit_qk(0, "wqr", qrT, nc.scalar)
                emit_qk(0, "wkr", krT, nc.scalar)
                emit_v(range(0, 4), vh)
                emit_qk(0, "wqi", qiT, nc.scalar)
                emit_v(range(4, 8), vh)
                emit_qk(0, "wki", kiT, nc.scalar)
                for wn, dstT in (
                    ("wqr", qrT),
                    ("wkr", krT),
                    ("wqi", qiT),
                    ("wki", kiT),
                ):
                    # pair-1 weights ride the sync queue (behind x + wy) so
                    # they don't clog the scalar queue ahead of the exp stream
                    emit_qk(1, wn, dstT, nc.sync)

                # ---------------- Phase B: attention ----------------
                with (
                    tc.tile_pool(name="pb_pt", bufs=6) as ptpool,
                    tc.tile_pool(name="pb_on", bufs=2) as onpool,
                    tc.tile_pool(name="pb_oav", bufs=18) as oavp,
                ):

                    def emit_qk_exp(pair, g, ic):
                        qT = qrT if g in (0, 1) else qiT
                        kT = krT if g in (0, 2) else kiT
                        ptA = ptpool.tile([P, 8, 512], BF16, tag="pt", name="ptA")
                        ptB = ptpool.tile([P, 8, 512], BF16, tag="pt", name="ptB")
                        for u in range(4):
                            stA = pst.tile([P, 1024], F32, tag="st", name="stA")
                            stB = pst.tile([P, 1024], F32, tag="st", name="stB")
                            for jj in range(2):
                                jt = 2 * u + jj
                                nc.tensor.matmul(
                                    stA[:, jj * 512 : (jj + 1) * 512],
                                    kT[0:CD, pair, jt * 128 : (jt + 1) * 128],
                                    qT[0:CD, pair, ic * 512 : (ic + 1) * 512],
                                    start=True,
                                    stop=True,
                                )
                                nc.tensor.matmul(
                                    stB[:, jj * 512 : (jj + 1) * 512],
                                    kT[CD:P, pair, jt * 128 : (jt + 1) * 128],
                                    qT[CD:P, pair, ic * 512 : (ic + 1) * 512],
                                    start=True,
                                    stop=True,
                                )
                            nc.scalar.activation(
                                ptA[:, 2 * u : 2 * u + 2, :].rearrange(
                                    "p a b -> p (a b)"
                                ),
                                stA[:],
                                EXP,
                                scale=SCALE,
                            )
                            nc.scalar.activation(
                                ptB[:, 2 * u : 2 * u + 2, :].rearrange(
                                    "p a b -> p (a b)"
                                ),
                                stB[:],
                                EXP,
                                scale=SCALE,
                            )
                        return ptA, ptB

                    def emit_avs(ps_, pts, pstate):
                        pair, g, ic = ps_
                        iu = g * 2 + ic
                        vh_ = vhat_r if g in (0, 2) else vhat_i
                        for hslot, pt in enumerate(pts):
                            h = pair * 2 + hslot
                            av = pav.tile([CD + 1, 512], F32, tag="av", name="av")
                            for jt in range(8):
                                nc.tensor.matmul(
                                    av[:],
                                    vh_[:, jt, h, :],
                                    pt[:, jt, :],
                                    start=(jt == 0),
                                    stop=(jt == 7),
                                )
                            oav = oavp.tile(
                                [CD + 1, 512], BF16, tag="oav", name="oav"
                            )
                            nc.vector.tensor_copy(oav[:], av[:])
                            nc.sync.dma_start(
                                pstate["den"][hslot * 8 + iu : hslot * 8 + iu + 1, :],
                                oav[CD : CD + 1, :],
                            )
                            pstate["oavs"].append((hslot, g, ic, oav))

                    def emit_norm_combine(pair, pstate):
                        rp = sp.tile([16, 512], BF16, tag="rp", name="rp")
                        with nc.allow_low_precision("bf16 softmax reciprocal"):
                            nc.vector.reciprocal(rp[:], pstate["den"][:])
                        ons = {}
                        for hslot in (0, 1):
                            ons[hslot] = onpool.tile(
                                [CD, HL, NTOK], BF16, tag="on", name="on"
                            )
                        for hslot, g, ic, oav in pstate["oavs"]:
                            iu = g * 2 + ic
                            rp1 = sp.tile([1, 512], BF16, tag="rp1", name="rp1")
                            nc.sync.dma_start(
                                rp1[:], rp[hslot * 8 + iu : hslot * 8 + iu + 1, :]
                            )
                            bc = sp.tile([CD, 512], BF16, tag="bc", name="bc")
                            nc.gpsimd.partition_broadcast(bc[:], rp1[:])
                            nc.vector.tensor_mul(
                                ons[hslot][:, g, ic * 512 : (ic + 1) * 512],
                                oav[0:CD, :],
                                bc[:],
                            )
                        for hslot in (0, 1):
                            h = pair * 2 + hslot
                            on = ons[hslot]
                            # o_r = (o0-o3)-(o1+o2), o_i = (o0-o3)+(o1+o2)
                            s = sp.tile([CD, NTOK], BF16, tag="cs", name="cs", bufs=1)
                            t = sp.tile([CD, NTOK], BF16, tag="ct", name="ct", bufs=1)
                            oi = sp.tile([CD, NTOK], BF16, tag="oi", name="oi", bufs=1)
                            nc.vector.tensor_sub(s[:], on[:, 0, :], on[:, 3, :])
                            nc.vector.tensor_add(t[:], on[:, 1, :], on[:, 2, :])
                            nc.vector.tensor_sub(O[0:CD, h, :], s[:], t[:])
                            nc.vector.tensor_add(oi[:], s[:], t[:])
                            nc.sync.dma_start(O[CD:P, h, :], oi[:])

                    passes = [
                        (pair, g, ic)
                        for pair in (0, 1)
                        for g in (0, 2, 1, 3)
                        for ic in (0, 1)
                    ]

                    def new_pstate():
                        return {
                            "den": sp.tile([16, 512], BF16, tag="den", name="den"),
                            "oavs": [],
                        }

                    pstates = {}
                    prev = None
                    for ps_ in passes:
                        pts = emit_qk_exp(*ps_)
                        if prev is not None:
                            ppair = prev[0][0]
                            if ppair not in pstates:
                                pstates[ppair] = new_pstate()
                            emit_avs(prev[0], prev[1], pstates[ppair])
                            if prev[0][1:] == (3, 1):
                                emit_norm_combine(ppair, pstates.pop(ppair))
                        prev = (ps_, pts)
                    ppair = prev[0][0]
                    if ppair not in pstates:
                        pstates[ppair] = new_pstate()
                    emit_avs(prev[0], prev[1], pstates[ppair])
                    emit_norm_combine(ppair, pstates.pop(ppair))

            # ---------------- Phase C: output projection ----------------
            with tc.tile_pool(name="pc_o", bufs=4) as cop:
                for ri, W in ((0, wyr_sb), (1, wyi_sb)):
                    for tt in range(8):
                        ps = pst.tile([P, NTOK], F32, tag="st", name="psy")
                        for oc in range(2):
                            for kt in range(HL):
                                nc.tensor.matmul(
                                    ps[:, oc * 512 : (oc + 1) * 512],
                                    O[:, kt, tt * 128 : (tt + 1) * 128],
                                    W[:, kt, oc * 512 : (oc + 1) * 512],
                                    start=(kt == 0),
                                    stop=(kt == HL - 1),
                                )
                        ys = cop.tile([P, NTOK], F32, tag="ys", name="ys")
                        if tt % 2 == 0:
                            nc.vector.tensor_copy(ys[:], ps[:])
                        else:
                            nc.scalar.copy(ys[:], ps[:])
                        nc.sync.dma_start(
                            yp[ri, tt * 128 : (tt + 1) * 128, :], ys[:]
                        )
    nc.compile()
    return nc


def _prep(inputs):
    f = np.float32
    xr = np.asarray(inputs["x_real"], f)
    xi = np.asarray(inputs["x_imag"], f)
    wq_r = np.asarray(inputs["wq_r"], f)
    wq_i = np.asarray(inputs["wq_i"], f)
    wkv_r = np.asarray(inputs["wkv_r"], f)
    wkv_i = np.asarray(inputs["wkv_i"], f)
    wout_r = np.asarray(inputs["wout_r"], f)
    wout_i = np.asarray(inputs["wout_i"], f)

    def pk(a):
        # [2048, C] stacked weight/x -> [p, kt, C] SBUF layout, bf16
        return np.ascontiguousarray(
            a.reshape(KD, P, -1).transpose(1, 0, 2).astype(BF16NP)
        )

    def pkpair(a):
        # [2048, 256] -> [pair, p, kt, 128]
        return np.ascontiguousarray(
            a.reshape(KD, P, 2, 128)
            .transpose(2, 1, 0, 3)
            .astype(BF16NP)
        )

    c = np.ascontiguousarray
    in_maps = []
    for core in range(8):
        b, hg = divmod(core, 4)
        c0 = hg * 256
        X = np.concatenate([xr[b].T, xi[b].T], axis=0)
        sl = slice(c0, c0 + 256)
        vsl = slice(1024 + c0, 1024 + c0 + 256)
        m = {
            "x": pk(X),
            "wqr": pkpair(np.concatenate([wq_r[sl].T, -wq_i[sl].T], axis=0)),
            "wqi": pkpair(np.concatenate([wq_i[sl].T, wq_r[sl].T], axis=0)),
            "wkr": pkpair(np.concatenate([wkv_r[sl].T, -wkv_i[sl].T], axis=0)),
            "wki": pkpair(np.concatenate([wkv_i[sl].T, wkv_r[sl].T], axis=0)),
            "wv": pk(
                np.concatenate(
                    [
                        np.concatenate([wkv_r[vsl].T, -wkv_i[vsl].T], axis=0),
                        np.concatenate([wkv_i[vsl].T, wkv_r[vsl].T], axis=0),
                    ],
                    axis=1,
                )
            ),
        }
        Wyr = np.empty((512, 1024), f)
        Wyi = np.empty((512, 1024), f)
        for h in range(HL):
            cols = slice(c0 + h * CD, c0 + (h + 1) * CD)
            Wyr[h * 128 : h * 128 + CD] = wout_r[:, cols].T
            Wyr[h * 128 + CD : (h + 1) * 128] = -wout_i[:, cols].T
            Wyi[h * 128 : h * 128 + CD] = wout_i[:, cols].T
            Wyi[h * 128 + CD : (h + 1) * 128] = wout_r[:, cols].T
        m["wyr"] = c(Wyr.reshape(HL, P, NTOK).transpose(1, 0, 2).astype(BF16NP))
        m["wyi"] = c(Wyi.reshape(HL, P, NTOK).transpose(1, 0, 2).astype(BF16NP))
        in_maps.append(m)
    return in_maps


def _get_nc():
    global _nc_cache
    if _nc_cache is None:
        _nc_cache = _build()
    return _nc_cache


def _assemble(results):
    y = np.zeros((2, 2, NTOK, 1024), np.float32)
    for core in range(8):
        b = core // 4
        y[:, b] += results[core]["ypart"]
    return y


def run(inputs, trace=False, **kwargs):
    nc = _get_nc()
    in_maps = _prep(inputs)
    res = run_bass_kernel_spmd(
        nc, in_maps, core_ids=list(range(8)), trace=trace, **kwargs
    )
    return _assemble(res.results), res


def kernel(**inputs) -> np.ndarray:
    y, _ = run(inputs)
    return y


# revision 18
# speedup vs baseline: 1.0446x; 1.0446x over previous
"""Trainium2 Bass kernel for nn_ComplexMultiheadAttention.

Problem (reference.py): complex multihead attention,
  B=2, N=1024, D=1024, HEADS=16, dim_head=64.
  q/k/v = complex linear projections of x = x_real + i*x_imag,
  4 softmax-attention combos g0..g3 over (q-part, k-part, v-part),
  sign-combined into o_real/o_imag, then a complex output projection.
  Output: [2, B, N, D] fp32 (real, imag).

Sharding (8 NeuronCores): core c = (b = c // 4) x (head group hg = c % 4,
4 heads each). Each core computes projections + attention + sign-combine
for its 4 heads and a partial output projection; the host unshards by
summing the 4 partials per batch.

Kernel design (v2, all-bf16, fully interleaved):
- Everything bf16 (fp32 PSUM accumulation). ~9e-3 rel err.
- The Tile scheduler breaks ready-ties by emission order, so the
  emission sequence IS the priority schedule: projection chains,
  attention passes, AV units, normalization, and output-projection
  chains are emitted hand-interleaved so ScalarE (exp, the pacing
  engine: ~143us of ACT) starts at ~18us and never starves, while the
  PE fills its gaps with projections / AV / output-projection work.
- QK^T via K=64 row-packed matmul PAIRS (tile_position (0,0)/(64,0) by
  base partition): two heads' S^T tiles per PE pass.
- Denominator via ones-column appended to V (AV row 64). Normalization
  split per pair: reciprocal of the first 4 passes' denominators
  (rows 0:8) happens mid-pair, only the last 4 passes' reciprocal +
  scale sit on the tail. bc/mul chains pipelined with deep pools.
- Phase C (output projection) split per PAIR: heads 0-1 contribution
  computed during pair-1 attention (PE slack) and DMA'd to DRAM;
  heads 2-3 contribution added on top with gpsimd DMA accum_op=add.
- PSUM: A/C chains 2x[128,512] + QK st 2x[128,1024] + AV 2x[65,512]
  = 8 banks, so all phases coexist.
"""

import numpy as np

import concourse.mybir as mybir
import concourse.tile as tile
from concourse import bacc
from concourse.bass_utils import run_bass_kernel_spmd

P = 128
NTOK = 1024  # tokens per batch
KD = 16  # k-tiles over the stacked 2048 contraction dim
CD = 64  # dim per head
HL = 4  # heads per core
F32 = mybir.dt.float32
BF16 = mybir.dt.bfloat16
BF16NP = mybir.dt.np(mybir.dt.bfloat16)
EXP = mybir.ActivationFunctionType.Exp
ADD = mybir.AluOpType.add
SCALE = float(CD) ** -0.5

# pass order within a pair: g asc by weight-availability (qr,kr first)
GORDER = ((0, 0), (0, 1), (2, 0), (2, 1), (1, 0), (1, 1), (3, 0), (3, 1))

_nc_cache = None


def _build():
    nc = bacc.Bacc("TRN2", target_bir_lowering=False, debug=False, num_devices=8)

    x = nc.declare_dram_parameter("x", [P, KD, NTOK], BF16, isOutput=False)
    wnames = ["wqr", "wkr", "wqi", "wki"]
    wd = {
        n: nc.declare_dram_parameter(n, [2, P, KD, 128], BF16, isOutput=False)
        for n in wnames
    }  # [pair, p, kt, cols]
    wv = nc.declare_dram_parameter("wv", [P, KD, 512], BF16, isOutput=False)
    wyr = nc.declare_dram_parameter("wyr", [P, HL, NTOK], BF16, isOutput=False)
    wyi = nc.declare_dram_parameter("wyi", [P, HL, NTOK], BF16, isOutput=False)
    yp0 = nc.declare_dram_parameter("ypart0", [2, NTOK, 1024], F32, isOutput=True)
    yp1 = nc.declare_dram_parameter("ypart1", [2, NTOK, 1024], F32, isOutput=True)

    with tile.TileContext(nc) as tc:
        with (
            tc.tile_pool(name="persist", bufs=1) as pp,
            tc.tile_pool(name="small", bufs=2) as sp,
            tc.tile_pool(name="ps_a", bufs=2, space="PSUM") as psa,
            tc.tile_pool(name="ps_st", bufs=2, space="PSUM") as pst,
            tc.tile_pool(name="ps_av", bufs=2, space="PSUM") as pav,
            tc.tile_pool(name="pa_x", bufs=1) as pax,
            tc.tile_pool(name="pa_w", bufs=1) as paw,
            tc.tile_pool(name="pb_pt", bufs=5) as ptpool,
            tc.tile_pool(name="pb_on", bufs=2) as onpool,
            tc.tile_pool(name="pb_oav", bufs=11) as oavp,
            tc.tile_pool(name="pc_o", bufs=4) as cop,
        ):
            qrT = pp.tile([P, 2, NTOK], BF16, tag="qrT")
            qiT = pp.tile([P, 2, NTOK], BF16, tag="qiT")
            krT = pp.tile([P, 2, NTOK], BF16, tag="krT")
            kiT = pp.tile([P, 2, NTOK], BF16, tag="kiT")
            vhat_r = pp.tile([P, 8, HL, CD + 1], BF16, tag="vhr")
            vhat_i = pp.tile([P, 8, HL, CD + 1], BF16, tag="vhi")
            # [o_r dims 0:64 | o_i dims 64:128, head, tok]
            O = pp.tile([P, HL, NTOK], BF16, tag="O")
            wyr_sb = pp.tile([P, HL, NTOK], BF16, tag="wyrs")
            wyi_sb = pp.tile([P, HL, NTOK], BF16, tag="wyis")

            nc.vector.memset(vhat_r[:, :, :, CD : CD + 1], 1.0)
            nc.vector.memset(vhat_i[:, :, :, CD : CD + 1], 1.0)

            xs = pax.tile([P, KD, NTOK], BF16, tag="xs")
            for kt in range(KD):
                nc.sync.dma_start(xs[:, kt, :], x[:, kt, :])
            nc.sync.dma_start(wyr_sb[:], wyr[:])
            nc.sync.dma_start(wyi_sb[:], wyi[:])

            qk_dst = {"wqr": qrT, "wqi": qiT, "wkr": krT, "wki": kiT}

            def emit_qk(pair, wn, dmaeng):
                dstT = qk_dst[wn]
                wt = paw.tile([P, KD, 128], BF16, tag="wt2", name="wt", bufs=2)
                for k0 in range(0, KD, 4):
                    dmaeng.dma_start(
                        wt[:, k0 : k0 + 4, :], wd[wn][pair, :, k0 : k0 + 4, :]
                    )
                for tch in range(2):
                    ps = psa.tile([P, 512], F32, tag="pa", name="ps")
                    for kt in range(KD):
                        nc.tensor.matmul(
                            ps[:],
                            wt[:, kt, :],
                            xs[:, kt, tch * 512 : (tch + 1) * 512],
                            start=(kt == 0),
                            stop=(kt == KD - 1),
                        )
                    nc.vector.tensor_copy(
                        dstT[:, pair, tch * 512 : (tch + 1) * 512], ps[:]
                    )

            vwt = [None]

            def emit_v(tts):
                if vwt[0] is None:
                    vwt[0] = paw.tile([P, KD, 512], BF16, tag="wtv", name="wtv", bufs=1)
                    for k0 in range(0, KD, 4):
                        nc.scalar.dma_start(
                            vwt[0][:, k0 : k0 + 4, :], wv[:, k0 : k0 + 4, :]
                        )
                for tt in tts:
                    ps = psa.tile([P, 512], F32, tag="pa", name="ps")
                    for kt in range(KD):
                        nc.tensor.matmul(
                            ps[:],
                            xs[:, kt, tt * 128 : (tt + 1) * 128],
                            vwt[0][:, kt, :],
                            start=(kt == 0),
                            stop=(kt == KD - 1),
                        )
                    nc.vector.tensor_copy(
                        vhat_r[:, tt, :, 0:CD],
                        ps[:, 0:256].rearrange("p (h d) -> p h d", d=CD),
                    )
                    nc.vector.tensor_copy(
                        vhat_i[:, tt, :, 0:CD],
                        ps[:, 256:512].rearrange("p (h d) -> p h d", d=CD),
                    )

            # ---------------- attention building blocks ----------------
            def emit_qk_exp(pair, g, ic):
                qT = qrT if g in (0, 1) else qiT
                kT = krT if g in (0, 2) else kiT
                ptA = ptpool.tile([P, 8, 512], BF16, tag="pt", name="ptA")
                ptB = ptpool.tile([P, 8, 512], BF16, tag="pt", name="ptB")
                for u in range(4):
                    stA = pst.tile([P, 1024], F32, tag="st", name="stA")
                    stB = pst.tile([P, 1024], F32, tag="st", name="stB")
                    for jj in range(2):
                        jt = 2 * u + jj
                        nc.tensor.matmul(
                            stA[:, jj * 512 : (jj + 1) * 512],
                            kT[0:CD, pair, jt * 128 : (jt + 1) * 128],
                            qT[0:CD, pair, ic * 512 : (ic + 1) * 512],
                            start=True,
                            stop=True,
                        )
                        nc.tensor.matmul(
                            stB[:, jj * 512 : (jj + 1) * 512],
                            kT[CD:P, pair, jt * 128 : (jt + 1) * 128],
                            qT[CD:P, pair, ic * 512 : (ic + 1) * 512],
                            start=True,
                            stop=True,
                        )
                    nc.scalar.activation(
                        ptA[:, 2 * u : 2 * u + 2, :].rearrange("p a b -> p (a b)"),
                        stA[:],
                        EXP,
                        scale=SCALE,
                    )
                    nc.scalar.activation(
                        ptB[:, 2 * u : 2 * u + 2, :].rearrange("p a b -> p (a b)"),
                        stB[:],
                        EXP,
                        scale=SCALE,
                    )
                return ptA, ptB

            def emit_avs(pair, pi, pts, pstate):
                g, ic = GORDER[pi]
                vh_ = vhat_r if g in (0, 2) else vhat_i
                for hslot, pt in enumerate(pts):
                    h = pair * 2 + hslot
                    av = pav.tile([CD + 1, 512], F32, tag="av", name="av")
                    for jt in range(8):
                        nc.tensor.matmul(
                            av[:],
                            vh_[:, jt, h, :],
                            pt[:, jt, :],
                            start=(jt == 0),
                            stop=(jt == 7),
                        )
                    oav = oavp.tile([CD + 1, 512], BF16, tag="oav", name="oav")
                    nc.vector.tensor_copy(oav[:], av[:])
                    row = (pi % 2) * 2 + hslot
                    nc.gpsimd.dma_start(
                        pstate["den"][pi // 2][row : row + 1, :], oav[CD : CD + 1, :]
                    )
                    pstate["oavs"].append((pi, hslot, oav))

            def emit_norm(pair, pstate, q):
                # quarter q: passes 2q, 2q+1 (its own 4-row den tile)
                rp = pstate["rp"][q]
                with nc.allow_low_precision("bf16 softmax reciprocal"):
                    nc.vector.reciprocal(rp[:], pstate["den"][q][:])
                for pi, hslot, oav in pstate["oavs"]:
                    if pi // 2 != q:
                        continue
                    g, ic = GORDER[pi]
                    row = (pi % 2) * 2 + hslot
                    rp1 = sp.tile([1, 512], BF16, tag="rp1", name="rp1", bufs=5)
                    nc.gpsimd.dma_start(rp1[:], rp[row : row + 1, :])
                    bc = sp.tile([CD, 512], BF16, tag="bc", name="bc", bufs=5)
                    nc.gpsimd.partition_broadcast(bc[:], rp1[:])
                    nc.vector.tensor_mul(
                        pstate["on"][hslot][:, g, ic * 512 : (ic + 1) * 512],
                        oav[0:CD, :],
                        bc[:],
                    )

            def emit_combine(pair, pstate):
                for hslot in (0, 1):
                    h = pair * 2 + hslot
                    on = pstate["on"][hslot]
                    # o_r = (o0-o3)-(o1+o2), o_i = (o0-o3)+(o1+o2)
                    s = sp.tile([CD, NTOK], BF16, tag="cs", name="cs", bufs=1)
                    t = sp.tile([CD, NTOK], BF16, tag="ct", name="ct", bufs=1)
                    oi = sp.tile([CD, NTOK], BF16, tag="oi", name="oi", bufs=1)
                    nc.vector.tensor_sub(s[:], on[:, 0, :], on[:, 3, :])
                    nc.vector.tensor_add(t[:], on[:, 1, :], on[:, 2, :])
                    nc.vector.tensor_sub(O[0:CD, h, :], s[:], t[:])
                    nc.vector.tensor_add(oi[:], s[:], t[:])
                    nc.sync.dma_start(O[CD:P, h, :], oi[:])

            def new_pstate():
                return {
                    "den": [
                        sp.tile([4, 512], BF16, tag="den", name="den", bufs=4)
                        for _ in range(4)
                    ],
                    "rp": [
                        sp.tile([4, 512], BF16, tag="rp", name="rp", bufs=4)
                        for _ in range(4)
                    ],
                    "on": {
                        0: onpool.tile([CD, HL, NTOK], BF16, tag="on", name="onA"),
                        1: onpool.tile([CD, HL, NTOK], BF16, tag="on", name="onB"),
                    },
                    "oavs": [],
                }

            # phase C chains, one (ri, tt, oc, kt-half) unit per call group
            def emit_c_chains(units, khalf, accum):
                for ri, tt, oc in units:
                    W = wyr_sb if ri == 0 else wyi_sb
                    ps = psa.tile([P, 512], F32, tag="pa", name="psy")
                    for j, kt in enumerate((khalf * 2, khalf * 2 + 1)):
                        nc.tensor.matmul(
                            ps[:],
                            O[:, kt, tt * 128 : (tt + 1) * 128],
                            W[:, kt, oc * 512 : (oc + 1) * 512],
                            start=(j == 0),
                            stop=(j == 1),
                        )
                    ys = cop.tile([P, 512], F32, tag="ys", name="ys")
                    if (tt + oc) % 2 == 0:
                        nc.vector.tensor_copy(ys[:], ps[:])
                    else:
                        nc.scalar.copy(ys[:], ps[:])
                    yph = yp1 if khalf else yp0
                    nc.sync.dma_start(
                        yph[ri, tt * 128 : (tt + 1) * 128, oc * 512 : (oc + 1) * 512],
                        ys[:],
                    )

            c_units = [(ri, tt, oc) for ri in range(2) for tt in range(8) for oc in range(2)]

            # ---------------- interleaved emission schedule ----------------
            # A-work injected before the QK of pass index i (global 0..15)
            pre_qk = {
                0: lambda: (emit_qk(0, "wqr", nc.scalar), emit_qk(0, "wkr", nc.scalar)),
                2: lambda: (emit_v(range(0, 4)), emit_qk(0, "wqi", nc.scalar)),
                4: lambda: (emit_v(range(4, 8)), emit_qk(0, "wki", nc.scalar)),
                6: lambda: emit_qk(1, "wqr", nc.sync),
                7: lambda: emit_qk(1, "wkr", nc.sync),
                8: lambda: emit_qk(1, "wqi", nc.sync),
                9: lambda: emit_qk(1, "wki", nc.sync),
            }

            passes = [(pair, pi) for pair in (0, 1) for pi in range(8)]
            pstates = {0: new_pstate(), 1: new_pstate()}
            # v chains must NOT be emitted between row-tiled QK passes
            # (hw race: full-row ldweights vs in-flight row-tiled matmuls)
            pre_qk.pop(0)()
            emit_v(range(0, 8))
            pre_qk[2] = lambda: emit_qk(0, "wqi", nc.scalar)
            pre_qk[4] = lambda: emit_qk(0, "wki", nc.scalar)

            # C-half-0 chains drip-fed after pair-1 AVs (PE slack in late B)
            cdrip = {9 + j: c_units[j * 5 : (j + 1) * 5] for j in range(7)}

            prev = None
            for gi, (pair, pi) in enumerate(passes):
                if gi in pre_qk:
                    pre_qk[gi]()
                pts = emit_qk_exp(pair, GORDER[pi][0], GORDER[pi][1])
                if prev is not None:
                    ppair, ppi = prev[0]
                    emit_avs(ppair, ppi, prev[1], pstates[ppair])
                    if ppi % 2 == 1:
                        emit_norm(ppair, pstates[ppair], ppi // 2)
                    if ppi == 7:
                        emit_combine(ppair, pstates[ppair])
                    if gi in cdrip:
                        emit_c_chains(cdrip[gi], 0, accum=False)
                prev = ((pair, pi), pts)
            ppair, ppi = prev[0]
            emit_avs(ppair, ppi, prev[1], pstates[ppair])
            emit_norm(ppair, pstates[ppair], 3)
            emit_combine(ppair, pstates[ppair])
            # leftover C-half-0 chains, then the accumulating half-1 sweep
            emit_c_chains(c_units[35:], 0, accum=False)
            emit_c_chains(c_units, 1, accum=False)
    nc.compile()
    return nc


def _prep(inputs):
    f = np.float32
    xr = np.asarray(inputs["x_real"], f)
    xi = np.asarray(inputs["x_imag"], f)
    wq_r = np.asarray(inputs["wq_r"], f)
    wq_i = np.asarray(inputs["wq_i"], f)
    wkv_r = np.asarray(inputs["wkv_r"], f)
    wkv_i = np.asarray(inputs["wkv_i"], f)
    wout_r = np.asarray(inputs["wout_r"], f)
    wout_i = np.asarray(inputs["wout_i"], f)

    def pk(a):
        # [2048, C] stacked weight/x -> [p, kt, C] SBUF layout, bf16
        return np.ascontiguousarray(
            a.reshape(KD, P, -1).transpose(1, 0, 2).astype(BF16NP)
        )

    def pkpair(a):
        # [2048, 256] -> [pair, p, kt, 128]
        return np.ascontiguousarray(
            a.reshape(KD, P, 2, 128).transpose(2, 1, 0, 3).astype(BF16NP)
        )

    c = np.ascontiguousarray
    in_maps = []
    for core in range(8):
        b, hg = divmod(core, 4)
        c0 = hg * 256
        X = np.concatenate([xr[b].T, xi[b].T], axis=0)
        sl = slice(c0, c0 + 256)
        vsl = slice(1024 + c0, 1024 + c0 + 256)
        m = {
            "x": pk(X),
            "wqr": pkpair(np.concatenate([wq_r[sl].T, -wq_i[sl].T], axis=0)),
            "wqi": pkpair(np.concatenate([wq_i[sl].T, wq_r[sl].T], axis=0)),
            "wkr": pkpair(np.concatenate([wkv_r[sl].T, -wkv_i[sl].T], axis=0)),
            "wki": pkpair(np.concatenate([wkv_i[sl].T, wkv_r[sl].T], axis=0)),
            "wv": pk(
                np.concatenate(
                    [
                        np.concatenate([wkv_r[vsl].T, -wkv_i[vsl].T], axis=0),
                        np.concatenate([wkv_i[vsl].T, wkv_r[vsl].T], axis=0),
                    ],
                    axis=1,
                )
            ),
        }
        Wyr = np.empty((512, 1024), f)
        Wyi = np.empty((512, 1024), f)
        for h in range(HL):
            cols = slice(c0 + h * CD, c0 + (h + 1) * CD)
            Wyr[h * 128 : h * 128 + CD] = wout_r[:, cols].T
            Wyr[h * 128 + CD : (h + 1) * 128] = -wout_i[:, cols].T
            Wyi[h * 128 : h * 128 + CD] = wout_i[:, cols].T
            Wyi[h * 128 + CD : (h + 1) * 128] = wout_r[:, cols].T
        m["wyr"] = c(Wyr.reshape(HL, P, NTOK).transpose(1, 0, 2).astype(BF16NP))
        m["wyi"] = c(Wyi.reshape(HL, P, NTOK).transpose(1, 0, 2).astype(BF16NP))
        in_maps.append(m)
    return in_maps


def _get_nc():
    global _nc_cache
    if _nc_cache is None:
        _nc_cache = _build()
    return _nc_cache


def _assemble(results):
    y = np.zeros((2, 2, NTOK, 1024), np.float32)
    for core in range(8):
        b = core // 4
        y[:, b] += results[core]["ypart0"]
        y[:, b] += results[core]["ypart1"]
    return y


def run(inputs, trace=False, **kwargs):
    nc = _get_nc()
    in_maps = _prep(inputs)
    res = run_bass_kernel_spmd(
        nc, in_maps, core_ids=list(range(8)), trace=trace, **kwargs
    )
    return _assemble(res.results), res


def kernel(**inputs) -> np.ndarray:
    y, _ = run(inputs)
    return y
